# revision 8
# baseline (speedup 1.0000x reference)
"""Trainium2 Bass kernel for nn_Net_73710228734901.

The network's post-gather graph (concat -> Conv3d -> spatial mean -> Linear)
is entirely linear in the gathered pixels, and the gathers / avg-pool /
1x1-conv are linear in the inputs.  Since the output is only [B, 1], the
whole model collapses to

    out[b] = lin_b + <W1, x1[b]> + <W2, x2[b]> + <W4, share[b]> + <W3, x3[b]>

with fixed per-element weight tensors W* computed (cheaply, on host) from
c_w / conv3d_w / lin_w / idx_h / idx_w.  The device kernel is then a pure
memory-bound weighted reduction over the big activations: stream x through
SBUF and run one fused DVE tensor_tensor_reduce (multiply + free-dim sum)
per batch row, followed by a ones-matmul partition reduction.

Sharding: channels are sharded 8 ways (x1/x2/share: 128 ch/core, x3:
160 ch/core) so the weight tensors are split, not replicated; every core
holds all 64 batches and produces per-batch partial sums which the host
adds.  Per-core HBM traffic = 51.4 MB of activations + 0.8 MB of weights,
which is the roofline for this problem.
"""

import numpy as np

import concourse.bacc as bacc
import concourse.mybir as mybir
from concourse.bass_utils import run_bass_kernel_spmd
from concourse.tile import TileContext

NCORES = 8
NB = 64           # full batch, all on every core (channel sharding)
F1 = 196          # 14*14 spatial positions (x1/x2/share shards: 128 ch)
F3 = 980          # x3 shard: 160 ch * 784 pos / 128 partitions
F_TOT = 3 * F1 + F3   # 1568 free elems per (partition, batch)
BLK = 2           # batches per DMA chunk (1.6 MB each)
XBUFS = 6         # x-tile double-buffer depth

_F32 = mybir.dt.float32


def _build_fold(c_w, conv3d_w, lin_w, lin_b, idx_h, idx_w):
    """Collapse conv3d+mean+linear into per-element weights (float64 host math).

    Returns Ws1, Ws2, Ws4: [1024, 196] and Ws3: [1280, 784] float32.
    """
    c_w = c_w.astype(np.float64)
    conv3d_w = conv3d_w.astype(np.float64)
    lin_w = lin_w.astype(np.float64)

    # W2[c = i*64+dd, kh, kw] = sum_{o,d,kd: 3d-4+kd=dd} lin_w[o*24+d] * conv3d_w[o,i,kd,kh,kw]
    W2 = np.zeros((1024, 3, 3), np.float64)
    o_idx = np.arange(32) * 24
    i_idx = np.arange(16) * 64
    for d in range(24):
        for kd in range(3):
            dd = 3 * d - 4 + kd
            if 0 <= dd < 64:
                W2[i_idx + dd] += np.einsum(
                    'o,oikl->ikl', lin_w[o_idx + d, 0], conv3d_w[:, :, kd])

    # Mean over the 14x14 conv output folds each (kh,kw) tap into a border mask.
    M = np.zeros((3, 3, 14, 14), np.float64)
    rng = {0: (0, 13), 1: (0, 14), 2: (1, 14)}
    for kh in range(3):
        for kw in range(3):
            r0, r1 = rng[kh]
            c0, c1 = rng[kw]
            M[kh, kw, r0:r1, c0:c1] = 1.0
    A = np.einsum('ckl,klrs->crs', W2, M) / 196.0   # [1024, 14, 14]

    # Scatter each quadrant's 7x7 weight into the source's 14x14 grid at the
    # per-channel crop offset (inverse of the gather).
    def scatter(Aq, ih, iw):
        n = Aq.shape[0]
        Ws = np.zeros((n, 14, 14), np.float64)
        ci = np.arange(n)[:, None, None]
        ri = (ih[:, None] + np.arange(7))[:, :, None]
        wi = (iw[:, None] + np.arange(7))[:, None, :]
        Ws[ci, ri, wi] = Aq
        return Ws

    Ws1 = scatter(A[:, 0:7, 0:7], idx_h[0], idx_w[0])
    Ws2 = scatter(A[:, 7:14, 0:7], idx_h[1], idx_w[1])
    Ws3c = scatter(A[:, 0:7, 7:14], idx_h[2], idx_w[2])
    Ws4 = scatter(A[:, 7:14, 7:14], idx_h[3], idx_w[3])

    # x3 path: pull the scattered weights back through the 1x1 conv ...
    Wpool = np.einsum('oc,ohw->chw', c_w, Ws3c)     # [1280, 14, 14]
    # ... and through avg_pool2d(5, stride 2, pad 2) (transposed scatter).
    Ws3 = np.zeros((1280, 28, 28), np.float64)
    for dh in range(-2, 3):
        for dw in range(-2, 3):
            hs = [h for h in range(14) if 0 <= 2 * h + dh < 28]
            ws = [w for w in range(14) if 0 <= 2 * w + dw < 28]
            H = [2 * h + dh for h in hs]
            W_ = [2 * w + dw for w in ws]
            Ws3[:, np.ix_(H, W_)[0], np.ix_(H, W_)[1]] += \
                Wpool[:, np.ix_(hs, ws)[0], np.ix_(hs, ws)[1]] / 25.0

    return (Ws1.reshape(1024, 196).astype(np.float32),
            Ws2.reshape(1024, 196).astype(np.float32),
            Ws4.reshape(1024, 196).astype(np.float32),
            Ws3.reshape(1280, 784).astype(np.float32))


def _build_bass(blk=BLK, xbufs=XBUFS, dma_split=1):
    nc = bacc.Bacc("TRN2")
    xin = nc.dram_tensor("xin", [128, NB, F_TOT], _F32, kind="ExternalInput")
    win = nc.dram_tensor("win", [128, F_TOT], _F32, kind="ExternalInput")
    linb = nc.dram_tensor("linb", [1, 1], _F32, kind="ExternalInput")
    out = nc.dram_tensor("out", [1, NB], _F32, kind="ExternalOutput")

    with TileContext(nc) as tc:
        with (
            tc.tile_pool(name="cpool", bufs=1) as cpool,
            tc.tile_pool(name="xpool", bufs=xbufs) as xpool,
            tc.tile_pool(name="spool", bufs=2) as spool,
            tc.tile_pool(name="apool", bufs=1) as apool,
            tc.tile_pool(name="ppool", bufs=1, space="PSUM") as ppool,
        ):
            wt = cpool.tile([128, F_TOT], _F32)
            nc.sync.dma_start(out=wt[:], in_=win[:, :])
            lb = cpool.tile([1, 1], _F32)
            nc.sync.dma_start(out=lb[:], in_=linb[:, :])
            ones = cpool.tile([128, 1], _F32)
            nc.gpsimd.memset(ones[:], 1.0)

            acc = apool.tile([128, NB], _F32)
            for blk_i in range(NB // blk):
                xt = xpool.tile([128, blk, F_TOT], _F32, tag="xt")
                if dma_split == 1:
                    nc.sync.dma_start(
                        out=xt[:], in_=xin[:, blk_i * blk:(blk_i + 1) * blk, :])
                else:
                    step = blk // dma_split
                    for s in range(dma_split):
                        lo = blk_i * blk + s * step
                        nc.sync.dma_start(
                            out=xt[:, s * step:(s + 1) * step, :],
                            in_=xin[:, lo:lo + step, :])
                for j in range(blk):
                    b = blk_i * blk + j
                    scr = spool.tile([128, F_TOT], _F32, tag="scr")
                    # Fused multiply + free-dim sum in one DVE pass:
                    # out = (in0 * 1.0) * in1, accum_out = sum(out).
                    nc.vector.scalar_tensor_tensor(
                        out=scr[:],
                        in0=xt[:, j, :],
                        scalar=1.0,
                        in1=wt[:],
                        op0=mybir.AluOpType.mult,
                        op1=mybir.AluOpType.mult,
                        accum_out=acc[:, b:b + 1],
                    )

            # Cross-partition sum of the per-(partition, batch) partials.
            ps = ppool.tile([1, NB], _F32)
            nc.tensor.matmul(ps[:], lhsT=ones[:], rhs=acc[:], start=True, stop=True)
            res = apool.tile([1, NB], _F32)
            nc.vector.tensor_scalar(
                res[:], ps[:], lb[:], None, mybir.AluOpType.add)
            nc.sync.dma_start(out=out[:, :], in_=res[:])
    nc.finalize()
    return nc


def _shard_inputs(x1, x2, x3, share_feature, Ws1, Ws2, Ws4, Ws3, lin_b):
    in_maps = []
    for m in range(NCORES):
        cs = slice(m * 128, (m + 1) * 128)
        cs3 = slice(m * 160, (m + 1) * 160)
        xin = np.concatenate([
            x1[:, cs].reshape(NB, 128, F1),
            x2[:, cs].reshape(NB, 128, F1),
            share_feature[:, cs].reshape(NB, 128, F1),
            x3[:, cs3].reshape(NB, 128, F3),
        ], axis=2)                                   # [64, 128, 1568]
        xin = np.ascontiguousarray(xin.transpose(1, 0, 2))  # [128, 64, 1568]
        win = np.concatenate([
            Ws1[cs].reshape(128, F1),
            Ws2[cs].reshape(128, F1),
            Ws4[cs].reshape(128, F1),
            Ws3[cs3].reshape(128, F3),
        ], axis=1)                                   # [128, 1568]
        linb = np.array([[lin_b[0] if m == 0 else 0.0]], np.float32)
        in_maps.append({'xin': xin, 'win': np.ascontiguousarray(win),
                        'linb': linb})
    return in_maps


def kernel(x1, x2, x3, share_feature, c_w, conv3d_w, lin_w, lin_b,
           idx_h, idx_w):
    Ws1, Ws2, Ws4, Ws3 = _build_fold(c_w, conv3d_w, lin_w, lin_b,
                                     idx_h, idx_w)
    in_maps = _shard_inputs(x1, x2, x3, share_feature,
                            Ws1, Ws2, Ws4, Ws3, lin_b)
    nc = _build_bass()
    res = run_bass_kernel_spmd(nc, in_maps, core_ids=list(range(NCORES)))
    parts = np.stack([r['out'][0] for r in res.results])      # [8, 64]
    return parts.sum(axis=0, dtype=np.float64).astype(np.float32).reshape(NB, 1)


# revision 19
# speedup vs baseline: 1.5984x; 1.5984x over previous
"""Trainium2 Bass kernel for nn_Net_73710228734901.

The network's post-gather graph (concat -> Conv3d -> spatial mean -> Linear)
is entirely linear in the gathered pixels, and the gathers / avg-pool /
1x1-conv are linear in the inputs.  Since the output is only [B, 1], the
whole model collapses to

    out[b] = lin_b + <W1, x1[b]> + <W2, x2[b]> + <W4, share[b]> + <W3, x3[b]>

with fixed per-element weight tensors W* computed (cheaply, on host) from
c_w / conv3d_w / lin_w / idx_h / idx_w.  The device kernel is then a pure
memory-bound weighted reduction over the big activations: stream x through
SBUF and run one fused DVE tensor_tensor_reduce (multiply + free-dim sum)
per batch row, followed by a ones-matmul partition reduction.

Sharding: channels are sharded 8 ways (x1/x2/share: 128 ch/core, x3:
160 ch/core) so the weight tensors are split, not replicated; every core
holds all 64 batches and produces per-batch partial sums which the host
adds.  Per-core HBM traffic = 51.4 MB of activations + 0.8 MB of weights,
which is the roofline for this problem.
"""

import numpy as np

import concourse.bacc as bacc
import concourse.mybir as mybir
from concourse.bass_utils import run_bass_kernel_spmd
from concourse.tile import TileContext

NCORES = 8
NB = 64           # full batch, all on every core (channel sharding)
F1 = 196          # 14*14 spatial positions (x1/x2/share shards: 128 ch)
F3 = 980          # x3 shard: 160 ch * 784 pos / 128 partitions
F_TOT = 3 * F1 + F3   # 1568 free elems per (partition, batch)
BLK = 4           # batches per DMA chunk (1.6 MB each in fp16)
XBUFS = 6         # x-tile double-buffer depth
ACT_NUM = 43      # of every 64 batches, this many take the TT+ACT path
ACT_W16 = True    # TT path reads fp16 weights (2x DVE mode) vs fp32
W_SCALE = 1024.0  # weights pre-scaled by 2^10 so fp16 products avoid
                  # subnormals; undone exactly in the final combine

_F32 = mybir.dt.float32
_F16 = mybir.dt.float16


def _build_fold(c_w, conv3d_w, lin_w, lin_b, idx_h, idx_w):
    """Collapse conv3d+mean+linear into per-element weights (float64 host math).

    Returns Ws1, Ws2, Ws4: [1024, 196] and Ws3: [1280, 784] float32.
    """
    c_w = c_w.astype(np.float64)
    conv3d_w = conv3d_w.astype(np.float64)
    lin_w = lin_w.astype(np.float64)

    # W2[c = i*64+dd, kh, kw] = sum_{o,d,kd: 3d-4+kd=dd} lin_w[o*24+d] * conv3d_w[o,i,kd,kh,kw]
    W2 = np.zeros((1024, 3, 3), np.float64)
    o_idx = np.arange(32) * 24
    i_idx = np.arange(16) * 64
    for d in range(24):
        for kd in range(3):
            dd = 3 * d - 4 + kd
            if 0 <= dd < 64:
                W2[i_idx + dd] += np.einsum(
                    'o,oikl->ikl', lin_w[o_idx + d, 0], conv3d_w[:, :, kd])

    # Mean over the 14x14 conv output folds each (kh,kw) tap into a border mask.
    M = np.zeros((3, 3, 14, 14), np.float64)
    rng = {0: (0, 13), 1: (0, 14), 2: (1, 14)}
    for kh in range(3):
        for kw in range(3):
            r0, r1 = rng[kh]
            c0, c1 = rng[kw]
            M[kh, kw, r0:r1, c0:c1] = 1.0
    A = np.einsum('ckl,klrs->crs', W2, M) / 196.0   # [1024, 14, 14]

    # Scatter each quadrant's 7x7 weight into the source's 14x14 grid at the
    # per-channel crop offset (inverse of the gather).
    def scatter(Aq, ih, iw):
        n = Aq.shape[0]
        Ws = np.zeros((n, 14, 14), np.float64)
        ci = np.arange(n)[:, None, None]
        ri = (ih[:, None] + np.arange(7))[:, :, None]
        wi = (iw[:, None] + np.arange(7))[:, None, :]
        Ws[ci, ri, wi] = Aq
        return Ws

    Ws1 = scatter(A[:, 0:7, 0:7], idx_h[0], idx_w[0])
    Ws2 = scatter(A[:, 7:14, 0:7], idx_h[1], idx_w[1])
    Ws3c = scatter(A[:, 0:7, 7:14], idx_h[2], idx_w[2])
    Ws4 = scatter(A[:, 7:14, 7:14], idx_h[3], idx_w[3])

    # x3 path: pull the scattered weights back through the 1x1 conv ...
    Wpool = np.einsum('oc,ohw->chw', c_w, Ws3c)     # [1280, 14, 14]
    # ... and through avg_pool2d(5, stride 2, pad 2) (transposed scatter).
    Ws3 = np.zeros((1280, 28, 28), np.float64)
    for dh in range(-2, 3):
        for dw in range(-2, 3):
            hs = [h for h in range(14) if 0 <= 2 * h + dh < 28]
            ws = [w for w in range(14) if 0 <= 2 * w + dw < 28]
            H = [2 * h + dh for h in hs]
            W_ = [2 * w + dw for w in ws]
            Ws3[:, np.ix_(H, W_)[0], np.ix_(H, W_)[1]] += \
                Wpool[:, np.ix_(hs, ws)[0], np.ix_(hs, ws)[1]] / 25.0

    return (Ws1.reshape(1024, 196).astype(np.float32),
            Ws2.reshape(1024, 196).astype(np.float32),
            Ws4.reshape(1024, 196).astype(np.float32),
            Ws3.reshape(1280, 784).astype(np.float32))


def _build_bass(blk=BLK, xbufs=XBUFS, act_num=ACT_NUM, act_w16=ACT_W16):
    """Per-batch weighted reduction, DMA-bound design.

    x streams in as fp16 (host-cast; halves HBM traffic); weights are
    fp32 (plus an optional fp16 copy when act_w16).  Each batch's
    multiply+sum runs on one of two engine paths so no single engine is
    the bottleneck:
      - STT path (DVE only): fused scalar_tensor_tensor (mult + accum)
      - TT+ACT path: DVE tensor_tensor product, then scalar-engine
        activation(Copy) whose accum_out does the free-dim sum
    act_num of every 64 batches take the TT+ACT path (Bresenham-spread).
    """
    nc = bacc.Bacc("TRN2")
    xin = nc.dram_tensor("xin", [128, NB, F_TOT], _F16, kind="ExternalInput")
    win = nc.dram_tensor("win", [128, F_TOT], _F32, kind="ExternalInput")
    linb = nc.dram_tensor("linb", [1, 1], _F32, kind="ExternalInput")
    out = nc.dram_tensor("out", [1, NB], _F32, kind="ExternalOutput")
    if act_w16:
        win16 = nc.dram_tensor("win16", [128, F_TOT], _F16,
                               kind="ExternalInput")

    with TileContext(nc) as tc:
        with (
            tc.tile_pool(name="cpool", bufs=1) as cpool,
            tc.tile_pool(name="xpool", bufs=xbufs) as xpool,
            tc.tile_pool(name="spool", bufs=2) as spool,
            tc.tile_pool(name="gpool", bufs=3) as gpool,
            tc.tile_pool(name="apool", bufs=1) as apool,
            tc.tile_pool(name="ppool", bufs=1, space="PSUM") as ppool,
        ):
            wt = cpool.tile([128, F_TOT], _F32)
            nc.sync.dma_start(out=wt[:], in_=win[:, :])
            if act_w16:
                wt16 = cpool.tile([128, F_TOT], _F16)
                nc.sync.dma_start(out=wt16[:], in_=win16[:, :])
            lb = cpool.tile([1, 1], _F32)
            nc.sync.dma_start(out=lb[:], in_=linb[:, :])
            ones = cpool.tile([128, 1], _F32)
            nc.gpsimd.memset(ones[:], 1.0)

            prod_dt = _F16 if act_w16 else _F32
            acc = apool.tile([128, NB], _F32)
            for blk_i in range(NB // blk):
                xt = xpool.tile([128, blk, F_TOT], _F16, tag="xt")
                nc.sync.dma_start(
                    out=xt[:], in_=xin[:, blk_i * blk:(blk_i + 1) * blk, :])
                for j in range(blk):
                    b = blk_i * blk + j
                    on_act = (b * act_num) % NB < act_num
                    if not on_act:
                        scr = spool.tile([128, F_TOT], _F32, tag="scr")
                        # Fused multiply + free-dim sum in one DVE pass:
                        # out = (in0 * 1.0) * in1, accum_out = sum(out).
                        nc.vector.scalar_tensor_tensor(
                            out=scr[:],
                            in0=xt[:, j, :],
                            scalar=1.0,
                            in1=wt[:],
                            op0=mybir.AluOpType.mult,
                            op1=mybir.AluOpType.mult,
                            accum_out=acc[:, b:b + 1],
                        )
                    else:
                        prod = gpool.tile([128, F_TOT], prod_dt, tag="prod")
                        nc.vector.tensor_tensor(
                            prod[:], xt[:, j, :],
                            wt16[:] if act_w16 else wt[:],
                            mybir.AluOpType.mult)
                        sink = gpool.tile([128, F_TOT], prod_dt, tag="sink")
                        nc.scalar.activation(
                            sink[:], prod[:],
                            mybir.ActivationFunctionType.Copy,
                            accum_out=acc[:, b:b + 1])

            # Cross-partition sum of the per-(partition, batch) partials,
            # then undo the weight pre-scale and add lin_b.
            ps = ppool.tile([1, NB], _F32)
            nc.tensor.matmul(ps[:], lhsT=ones[:], rhs=acc[:], start=True, stop=True)
            res = apool.tile([1, NB], _F32)
            nc.vector.tensor_scalar(
                res[:], ps[:], 1.0 / W_SCALE, lb[:],
                mybir.AluOpType.mult, mybir.AluOpType.add)
            nc.sync.dma_start(out=out[:, :], in_=res[:])
    nc.finalize()
    return nc


def _shard_inputs(x1, x2, x3, share_feature, Ws1, Ws2, Ws4, Ws3, lin_b,
                  include_w16=ACT_W16):
    in_maps = []
    for m in range(NCORES):
        cs = slice(m * 128, (m + 1) * 128)
        cs3 = slice(m * 160, (m + 1) * 160)
        xin = np.concatenate([
            x1[:, cs].reshape(NB, 128, F1),
            x2[:, cs].reshape(NB, 128, F1),
            share_feature[:, cs].reshape(NB, 128, F1),
            x3[:, cs3].reshape(NB, 128, F3),
        ], axis=2)                                   # [64, 128, 1568]
        xin = np.ascontiguousarray(
            xin.transpose(1, 0, 2), dtype=np.float16)  # [128, 64, 1568] fp16
        win = np.concatenate([
            Ws1[cs].reshape(128, F1),
            Ws2[cs].reshape(128, F1),
            Ws4[cs].reshape(128, F1),
            Ws3[cs3].reshape(128, F3),
        ], axis=1)                                   # [128, 1568]
        linb = np.array([[lin_b[0] if m == 0 else 0.0]], np.float32)
        win = np.ascontiguousarray(win * W_SCALE, dtype=np.float32)
        im = {'xin': xin, 'win': win, 'linb': linb}
        if include_w16:
            im['win16'] = win.astype(np.float16)
        in_maps.append(im)
    return in_maps


def _ensure_ntff_hook():
    """Make `trace=True` (e.g. BASS_TRACE=1) work under axon even when the
    image's antenv package lacks axon_hooks: register an equivalent module
    backed by the ctypes NTFF hook from trn_agent_boot."""
    import sys
    import types
    try:
        import antenv.axon_hooks  # noqa: F401
        return
    except Exception:
        pass
    try:
        from trn_agent_boot import trn_boot
        hook = trn_boot._ntff_profile_via_ctypes('/opt/axon/libaxon_pjrt.so')
        mod = types.ModuleType('antenv.axon_hooks')
        mod.get_axon_ntff_profile_hook = lambda: hook
        mod.set_axon_ntff_profile_hook = lambda h: None
        sys.modules['antenv.axon_hooks'] = mod
    except Exception:
        pass


def kernel(x1, x2, x3, share_feature, c_w, conv3d_w, lin_w, lin_b,
           idx_h, idx_w):
    x1, x2, x3 = np.asarray(x1), np.asarray(x2), np.asarray(x3)
    share_feature = np.asarray(share_feature)
    c_w, conv3d_w = np.asarray(c_w), np.asarray(conv3d_w)
    lin_w, lin_b = np.asarray(lin_w), np.asarray(lin_b)
    idx_h, idx_w = np.asarray(idx_h), np.asarray(idx_w)
    _ensure_ntff_hook()
    Ws1, Ws2, Ws4, Ws3 = _build_fold(c_w, conv3d_w, lin_w, lin_b,
                                     idx_h, idx_w)
    in_maps = _shard_inputs(x1, x2, x3, share_feature,
                            Ws1, Ws2, Ws4, Ws3, lin_b)
    nc = _build_bass()
    res = run_bass_kernel_spmd(nc, in_maps, core_ids=list(range(NCORES)))
    parts = np.stack([r['out'][0] for r in res.results])      # [8, 64]
    return parts.sum(axis=0, dtype=np.float64).astype(np.float32).reshape(NB, 1)
